# revision 15
# baseline (speedup 1.0000x reference)
"""TP-8 Trainium2 Bass kernel for a LLaDA/Llama transformer block.

Shapes (hardcoded): x [2, 1024, 4096], 32 heads x 128 head_dim,
FF=12288, non-causal attention, RMSNorm + RoPE + SwiGLU.

Sharding (per sharding_hint): tensor-parallel over the 8 cores —
q/k/v/ff sharded on the output-feature axis (4 heads / 1536 ff dims per
core), wo/w_out sharded on the contraction axis.  One bf16 on-device
AllReduce per batch restores the residual stream after attention; the
final projection partials are summed on the host.

v2 schedule (vs the fp16 v1 baseline at 2.58 ms):
 - per-batch pipeline: qkv(b0) attn(b0) oproj(b0)+AR(b0) qkv... so each
   batch's AllReduce overlaps the other batch's compute / the MLP.
 - bf16 operands (FWL-eligible weight loads), consecutive matmuls share
   a stationary via ch-inner loop order.
 - x->xn shares one SBUF tile ring; xmid tiles are shared between the
   norm2 stats, the ff/up moving operands and the final residual.
 - norm scale rs = exp(-0.5*ln(mean_sq + eps)); norm weights folded into
   the adjacent weight matrices on the host; 1/sqrt(head_dim) in wq.
 - cross-partition sums (sum over D, softmax denominator) use an
   all-ones stationary operand; softmax needs no max subtraction.
"""

from contextlib import ExitStack

import numpy as np
import ml_dtypes

import concourse.mybir as mybir
import concourse.tile as tile
from concourse import bacc
from concourse.bass_utils import run_bass_kernel_spmd

F32 = mybir.dt.float32
DT = mybir.dt.bfloat16
NPDT = ml_dtypes.bfloat16
AF = mybir.ActivationFunctionType
ALU = mybir.AluOpType

N_CORES = 8
P = 128
B, T, D, FF = 2, 1024, 4096, 12288
M = B * T            # 2048 tokens
H = 128              # head dim
HALF = 64
QC = D // N_CORES    # 512 per-core q/k/v features (4 heads)
NH = QC // H         # 4 heads per core
FC = FF // N_CORES   # 1536 per-core ff features
NKP = D // P         # 32 K-tiles over D
NFT = FC // P        # 12 M-tiles over per-core FF
NDT = D // P         # 32 D-tiles
NST = T // P         # 8 sequence tiles per batch
EPS = 1e-05


def _dedup_ldweights(nc):
    """Drop InstLdweights that reload the PE stationary already in the array.

    Tile lowering emits one InstLdweights per matmul even when consecutive
    matmuls share the stationary operand (our loops order ch-chunks inside
    the weight-tile loop exactly so they do).  A repeated load is pure
    overhead (~107 ns of PE weight-path time per matmul).  Safe rule: drop a
    PE InstLdweights iff it has no sems attached, its access pattern equals
    the previous PE weight-load with no intervening weight-state change, and
    the matmul it feeds has <=1 waits (so compile() won't hoist excess waits
    onto an earlier-kept load).
    """
    for blk in nc.m.functions[0].blocks:
        il = blk.instructions
        # wait-count of the next matmul after each position
        nxt_mm_waits = [0] * len(il)
        w = 0
        for i in range(len(il) - 1, -1, -1):
            ins = il[i]
            if isinstance(ins, mybir.InstMatmult):
                si = ins.sync_info
                w = len(si.on_wait) if si else 0
            nxt_mm_waits[i] = w
        out = []
        last_key = None
        n_drop = 0
        for i, ins in enumerate(il):
            if str(ins.engine) != "EngineType.PE":
                out.append(ins)
                continue
            if isinstance(ins, mybir.InstLdweights):
                ap = ins.ins[0]
                key = (
                    ap.memref, ap.offset, str(ap.ap), str(ap.dtype),
                    ins.is_transpose, ins.perf_mode, ins.tile_position,
                    ins.tile_size,
                )
                si = ins.sync_info
                clean = not si or (not si.on_wait and not si.on_update)
                if key == last_key and clean and nxt_mm_waits[i] <= 1:
                    n_drop += 1
                    continue
                last_key = key
                out.append(ins)
            elif isinstance(ins, mybir.InstMatmult):
                if ins.is_transpose:
                    last_key = None
                out.append(ins)
            elif isinstance(ins, mybir.InstEventSemaphore):
                out.append(ins)
            else:
                last_key = None
                out.append(ins)
        if n_drop:
            blk.instructions = out


def _thin_pe_sem_incs(nc):
    """Drop unobserved per-matmul semaphore increments.

    Tile's clock protocol attaches a +1 on the PE proc-sem to every matmul,
    but only a few hundred positions are ever referenced by a wait.  Each
    EVT_SEM write costs ~26 ns of serialized sequencer time.  Keep exactly
    the referenced positions (plus the final one), renumber the waits to
    their rank among kept increments — semantics are unchanged: every wait
    is satisfied at the same instruction as before.
    """
    from collections import defaultdict

    f = nc.m.functions[0]
    upd = defaultdict(list)   # sem id -> [(inst, update)] in program order
    waits = defaultdict(list)
    for blk in f.blocks:
        for ins in blk.instructions:
            si = ins.sync_info
            if not si:
                continue
            for u in si.on_update:
                if u.sync_type == "semaphore":
                    upd[u.id].append((ins, u))
            for w in si.on_wait:
                if w.sync_type == "semaphore":
                    waits[w.id].append(w)

    rank_by_sem = {}
    for sid, ups in upd.items():
        if len(ups) < 1000:
            continue
        if not all(isinstance(ins, mybir.InstMatmult) for ins, _ in ups):
            continue
        if not all(
            u.update_mode == "sem-inc" and u.update_value == 1 for _, u in ups
        ):
            continue
        ws = waits[sid]
        if not all(w.wait_mode == "sem-ge-imm" and w.wait_reg is None for w in ws):
            continue
        n = len(ups)
        if any(w.wait_value > n or w.wait_value < 1 for w in ws):
            continue
        kept = sorted(set(w.wait_value for w in ws) | {n})
        rank_by_sem[sid] = {v: i + 1 for i, v in enumerate(kept)}
        keptset = set(kept)
        for pos, (ins, u) in enumerate(ups, start=1):
            if pos not in keptset:
                si = ins.sync_info
                si.on_update = [x for x in si.on_update if x is not u]

    if not rank_by_sem:
        return
    for blk in f.blocks:
        for ins in blk.instructions:
            si = ins.sync_info
            if not si or not si.on_wait:
                continue
            changed = False
            for w in si.on_wait:
                if w.sync_type == "semaphore" and w.id in rank_by_sem:
                    w.wait_value = rank_by_sem[w.id][w.wait_value]
                    changed = True
            if changed:
                si.on_wait = si.on_wait


def _build():
    nc = bacc.Bacc("TRN2", target_bir_lowering=False, num_devices=N_CORES)

    xT_h = nc.declare_dram_parameter("xT_h", [D, M], DT, isOutput=False)
    css = nc.declare_dram_parameter("css", [2, P, M], DT, isOutput=False)
    wq_t = nc.declare_dram_parameter("wq_t", [NH, P, NKP, P], DT, isOutput=False)
    wk_t = nc.declare_dram_parameter("wk_t", [NH, P, NKP, P], DT, isOutput=False)
    wv_n = nc.declare_dram_parameter("wv_n", [D, QC], DT, isOutput=False)
    wo_t = nc.declare_dram_parameter("wo_t", [NH, P, NDT, P], DT, isOutput=False)
    wf_t = nc.declare_dram_parameter("wf_t", [NFT, P, NKP, P], DT, isOutput=False)
    wu_t = nc.declare_dram_parameter("wu_t", [NFT, P, NKP, P], DT, isOutput=False)
    wout_t = nc.declare_dram_parameter("wout_t", [NDT, P, NFT, P], DT, isOutput=False)
    y = nc.declare_dram_parameter("y", [D, M], F32, isOutput=True)

    with tile.TileContext(nc) as tc:
        _emit(nc, tc, xT_h, css, wq_t, wk_t, wv_n, wo_t, wf_t, wu_t, wout_t, y)
    _dedup_ldweights(nc)
    nc.compile()
    return nc


def _emit(nc, tc, xT_h, css, wq_t, wk_t, wv_n, wo_t, wf_t, wu_t, wout_t, y):
    with ExitStack() as top:
        dram_pool = top.enter_context(tc.tile_pool(name="dram", bufs=1, space="DRAM"))
        const = top.enter_context(tc.tile_pool(name="const", bufs=1))

        cc_in = [dram_pool.tile([D, T], DT, name=f"cc_in_{b}") for b in range(B)]
        cc_out = [
            dram_pool.tile([D, T], DT, addr_space="Shared", name=f"cc_out_{b}")
            for b in range(B)
        ]

        ones_h = const.tile([P, P], DT)
        nc.vector.memset(ones_h[:], 1.0)
        cc_sb = const.tile([P, M], DT)
        ss_sb = const.tile([P, M], DT)
        nc.sync.dma_start(out=cc_sb[:], in_=css[0])
        nc.sync.dma_start(out=ss_sb[:], in_=css[1])
        eps_sb = const.tile([P, 1], F32)
        nc.vector.memset(eps_sb[:], EPS)

        # ---------- attention + o-proj per batch; AR fires per batch ----------
        # xmid staging pool shared by both MLP halves (bufs=1 per kp-tag:
        # half 1's loads reuse half 0's slots as the wout loop frees them)
        xmp = top.enter_context(tc.tile_pool(name="xmid", bufs=1))
        xmh_all = []
        for b in range(B):
            bs = slice(b * T, (b + 1) * T)
            with ExitStack() as bph:
                bp = bph.enter_context(tc.tile_pool(name=f"bat_{b}", bufs=1))
                qf, kf, v_sb = [], [], []
                with ExitStack() as ph:
                    sp = ph.enter_context(tc.tile_pool(name=f"qkv_{b}", bufs=1))
                    # ---- rms-norm stats for this batch ----
                    xs = []
                    for kp in range(NKP):
                        xt = sp.tile([P, T], DT, tag="xs", bufs=33, name=f"x_{b}_{kp}")
                        nc.sync.dma_start(out=xt[:], in_=xT_h[kp * P : (kp + 1) * P, bs])
                        xs.append(xt)
                    bcast1 = sp.tile([P, T], DT, name=f"bc1_{b}")
                    with ExitStack() as sph:
                        spp = sph.enter_context(
                            tc.tile_pool(name=f"st_ps_{b}", bufs=1, space="PSUM")
                        )
                        ms_ps = spp.tile([P, T], F32, name=f"ms_ps_{b}")
                        for kp in range(NKP):
                            sq = sp.tile([P, T], DT, tag="sq", bufs=2, name=f"sq_{b}_{kp}")
                            if kp % 2 == 0:
                                nc.scalar.activation(sq[:], xs[kp][:], AF.Square)
                            else:
                                nc.vector.tensor_mul(sq[:], xs[kp][:], xs[kp][:])
                            for ch in range(T // 512):
                                cs = slice(ch * 512, (ch + 1) * 512)
                                nc.tensor.matmul(
                                    ms_ps[:, cs], ones_h[:], sq[:, cs],
                                    start=(kp == 0), stop=(kp == NKP - 1),
                                )
                        lnt = sp.tile([P, T], F32, name=f"lnt_{b}")
                        nc.scalar.activation(
                            lnt[:], ms_ps[:], AF.Ln, bias=eps_sb[:], scale=1.0 / D
                        )
                        nc.scalar.activation(bcast1[:], lnt[:], AF.Exp, scale=-0.5)
                    # xn tiles recycle the x ring slots
                    xn = []
                    for kp in range(NKP):
                        xnk = sp.tile([P, T], DT, tag="xs", bufs=33, name=f"xn_{b}_{kp}")
                        nc.vector.tensor_mul(xnk[:], xs[kp][:], bcast1[:])
                        xn.append(xnk)

                    # ---- q/k projections, rope fused into the eviction ----
                    with ExitStack() as qph:
                        qpp = qph.enter_context(
                            tc.tile_pool(name=f"qk_ps_{b}", bufs=1, space="PSUM")
                        )
                        for which, wsrc, dst in (("q", wq_t, qf), ("k", wk_t, kf)):
                            for m in range(NH):
                                wt = sp.tile(
                                    [P, NKP, P], DT, tag="wqk", bufs=2,
                                    name=f"w{which}_{b}_{m}",
                                )
                                nc.sync.dma_start(out=wt[:], in_=wsrc[m])
                                ps = qpp.tile(
                                    [P, T], F32, tag="qk_ps", bufs=2,
                                    name=f"ps{which}_{b}_{m}",
                                )
                                for kp in range(NKP):
                                    for ch in range(T // 512):
                                        cs = slice(ch * 512, (ch + 1) * 512)
                                        nc.tensor.matmul(
                                            ps[:, cs], wt[:, kp, :], xn[kp][:, cs],
                                            start=(kp == 0), stop=(kp == NKP - 1),
                                        )
                                main = sp.tile(
                                    [P, T], DT, tag="rmain", bufs=2,
                                    name=f"rm_{which}_{b}_{m}",
                                )
                                nc.vector.scalar_tensor_tensor(
                                    main[:], ps[:], 1.0, cc_sb[:, bs],
                                    ALU.mult, ALU.mult,
                                )
                                rot = sp.tile(
                                    [P, T], DT, tag="rrot", bufs=2,
                                    name=f"rr_{which}_{b}_{m}",
                                )
                                nc.vector.scalar_tensor_tensor(
                                    rot[:HALF], ps[HALF:], -1.0,
                                    ss_sb[:HALF, bs], ALU.mult, ALU.mult,
                                )
                                nc.vector.scalar_tensor_tensor(
                                    rot[HALF:], ps[:HALF], 1.0,
                                    ss_sb[HALF:, bs], ALU.mult, ALU.mult,
                                )
                                out = bp.tile(
                                    [P, T], DT, tag=f"{which}f{m}",
                                    name=f"{which}f_{b}_{m}",
                                )
                                nc.vector.tensor_add(out[:], main[:], rot[:])
                                dst.append(out)

                    # ---- v projection, token-major: 8 concurrent accumulators
                    with ExitStack() as vph:
                        vpp = vph.enter_context(
                            tc.tile_pool(name=f"v_ps_{b}", bufs=1, space="PSUM")
                        )
                        ps_v = [
                            vpp.tile([P, QC], F32, tag=f"vps{st}", name=f"psv_{b}_{st}")
                            for st in range(NST)
                        ]
                        for kp in range(NKP):
                            wvk = sp.tile(
                                [P, QC], DT, tag="wv", bufs=3, name=f"wv_{b}_{kp}"
                            )
                            nc.sync.dma_start(
                                out=wvk[:], in_=wv_n[kp * P : (kp + 1) * P, :]
                            )
                            for st in range(NST):
                                nc.tensor.matmul(
                                    ps_v[st][:],
                                    xn[kp][:, st * P : (st + 1) * P],
                                    wvk[:],
                                    start=(kp == 0), stop=(kp == NKP - 1),
                                )
                        for st in range(NST):
                            vt = bp.tile([P, QC], DT, tag=f"v{st}", name=f"v_{b}_{st}")
                            nc.scalar.copy(vt[:], ps_v[st][:])
                            v_sb.append(vt)

                # o-proj weights: pool opened before the attention pools so
                # its SBUF region doesn't overlap them — the loads prefetch
                # during attention instead of blocking the DMA queues.
                op_sp = bph.enter_context(tc.tile_pool(name=f"op_{b}", bufs=1))
                wo_sb = []
                for h in range(NH):
                    wt = op_sp.tile([P, NDT, P], DT, tag=f"wo{h}", name=f"wo_{b}_{h}")
                    nc.sync.dma_start(out=wt[:], in_=wo_t[h])
                    wo_sb.append(wt)

                # ---- attention per head ----
                attnf = []
                afp = bph.enter_context(tc.tile_pool(name=f"attnf_{b}", bufs=1))
                with ExitStack() as ah:
                    ap_ = ah.enter_context(tc.tile_pool(name=f"att_{b}", bufs=1))
                    app = ah.enter_context(
                        tc.tile_pool(name=f"att_ps_{b}", bufs=1, space="PSUM")
                    )
                    for h in range(NH):
                        den_ps = app.tile([P, T], F32, tag="den", name=f"den_{b}_{h}")
                        at_ps = app.tile([P, T], F32, tag="at", name=f"at_{b}_{h}")

                        def emit_lg(st):
                            lg_ps = app.tile(
                                [P, T], F32, tag="lg", bufs=2, name=f"lg_{b}_{h}_{st}"
                            )
                            for ch in range(T // 512):
                                cs = slice(ch * 512, (ch + 1) * 512)
                                nc.tensor.matmul(
                                    lg_ps[:, cs],
                                    kf[h][:, st * P : (st + 1) * P],
                                    qf[h][:, cs],
                                    start=True, stop=True,
                                )
                            pr = ap_.tile(
                                [P, T], DT, tag="probs", bufs=4,
                                name=f"pr_{b}_{h}_{st}",
                            )
                            for ch in range(T // 512):
                                cs = slice(ch * 512, (ch + 1) * 512)
                                nc.scalar.activation(pr[:, cs], lg_ps[:, cs], AF.Exp)
                            return pr

                        # software-pipelined: logits/exp of st+1 are emitted
                        # before den/pv of st so the PE has work during exp
                        prs = [None] * NST
                        prs[0] = emit_lg(0)
                        for st in range(NST):
                            if st + 1 < NST:
                                prs[st + 1] = emit_lg(st + 1)
                            pr = prs[st]
                            for ch in range(T // 512):
                                cs = slice(ch * 512, (ch + 1) * 512)
                                nc.tensor.matmul(
                                    den_ps[:, cs], ones_h[:], pr[:, cs],
                                    start=(st == 0), stop=(st == NST - 1),
                                )
                            for ch in range(T // 512):
                                cs = slice(ch * 512, (ch + 1) * 512)
                                nc.tensor.matmul(
                                    at_ps[:, cs],
                                    v_sb[st][:, h * H : (h + 1) * H],
                                    pr[:, cs],
                                    start=(st == 0), stop=(st == NST - 1),
                                )
                        # 1/den via ACT ln+exp: ~4x faster than DVE reciprocal
                        # and off the PE-resume critical path
                        af = afp.tile([P, T], DT, tag=f"af{h}", name=f"af_{b}_{h}")
                        for ch in range(T // 512):
                            cs = slice(ch * 512, (ch + 1) * 512)
                            rln = ap_.tile(
                                [P, 512], F32, tag="rln", bufs=4,
                                name=f"rln_{b}_{h}_{ch}",
                            )
                            nc.scalar.activation(rln[:], den_ps[:, cs], AF.Ln)
                            rec = ap_.tile(
                                [P, 512], F32, tag="rec", bufs=4,
                                name=f"rec_{b}_{h}_{ch}",
                            )
                            nc.scalar.activation(rec[:], rln[:], AF.Exp, scale=-1.0)
                            nc.vector.scalar_tensor_tensor(
                                af[:, cs], at_ps[:, cs], 1.0, rec[:],
                                ALU.mult, ALU.mult,
                            )
                        attnf.append(af)

                # ---- o-projection partial for this batch ----
                # dt-pairs with h outer: the first 12 matmuls of each pair
                # need only heads 0-2, covering the last head's eviction
                with ExitStack() as ph:
                    sp = ph.enter_context(tc.tile_pool(name=f"opx_{b}", bufs=1))
                    pp = ph.enter_context(
                        tc.tile_pool(name=f"op_ps_{b}", bufs=1, space="PSUM")
                    )
                    for dtb in range(NDT // 2):
                        pss = [
                            pp.tile([P, T], F32, tag="o_ps", bufs=4,
                                    name=f"pso_{b}_{dtb}_{j}")
                            for j in range(2)
                        ]
                        for h in range(NH):
                            for j in range(2):
                                dt = dtb * 2 + j
                                for ch in range(T // 512):
                                    cs = slice(ch * 512, (ch + 1) * 512)
                                    nc.tensor.matmul(
                                        pss[j][:, cs], wo_sb[h][:, dt, :],
                                        attnf[h][:, cs],
                                        start=(h == 0), stop=(h == NH - 1),
                                    )
                        for j in range(2):
                            dt = dtb * 2 + j
                            xt = sp.tile(
                                [P, T], DT, tag="xs3", bufs=3, name=f"xo_{b}_{dt}"
                            )
                            nc.sync.dma_start(
                                out=xt[:], in_=xT_h[dt * P : (dt + 1) * P, bs]
                            )
                            osb = sp.tile(
                                [P, T], DT, tag="osb", bufs=3, name=f"osb_{b}_{dt}"
                            )
                            nc.vector.scalar_tensor_tensor(
                                osb[:], xt[:], 1.0 / N_CORES, pss[j][:],
                                ALU.mult, ALU.add,
                            )
                            nc.sync.dma_start(
                                out=cc_in[b][dt * P : (dt + 1) * P, :], in_=osb[:]
                            )
                    nc.gpsimd.collective_compute(
                        "AllReduce",
                        ALU.add,
                        replica_groups=[list(range(N_CORES))],
                        ins=[cc_in[b][:, :]],
                        outs=[cc_out[b][:, :]],
                    )

            # xmid tiles for this batch's MLP half: loaded via the (otherwise
            # idle) GpSimd DMA path right after the AllReduce trigger, so the
            # AR-gated reads never block the HWDGE queues streaming weights.
            xm_list = []
            for kp in range(NKP):
                xk = xmp.tile([P, T], DT, tag=f"xm{kp}", name=f"xmh_{b}_{kp}")
                nc.gpsimd.dma_start(
                    out=xk[:], in_=cc_out[b][kp * P : (kp + 1) * P, :]
                )
                xm_list.append(xk)
            xmh_all.append(xm_list)

        # ---------------- SwiGLU MLP per batch-half ----------------
        for hb in range(B):
            bs = slice(hb * T, (hb + 1) * T)
            with ExitStack() as bph:
                bp = bph.enter_context(tc.tile_pool(name=f"mlpb_{hb}", bufs=1))
                hsb = []
                xmh = xmh_all[hb]
                with ExitStack() as ph:
                    sp = ph.enter_context(tc.tile_pool(name=f"mlp_{hb}", bufs=1))
                    bcast2 = sp.tile([P, T], DT, name=f"bc2_{hb}")
                    with ExitStack() as sph:
                        spp = sph.enter_context(
                            tc.tile_pool(name=f"st2_ps_{hb}", bufs=1, space="PSUM")
                        )
                        ms2 = spp.tile([P, T], F32, name=f"ms2_{hb}")
                        for kp in range(NKP):
                            sq = sp.tile(
                                [P, T], DT, tag="sq2", bufs=3, name=f"sq2_{hb}_{kp}"
                            )
                            if kp % 2 == 0:
                                nc.scalar.activation(sq[:], xmh[kp][:], AF.Square)
                            else:
                                nc.vector.tensor_mul(sq[:], xmh[kp][:], xmh[kp][:])
                            for ch in range(T // 512):
                                cs = slice(ch * 512, (ch + 1) * 512)
                                nc.tensor.matmul(
                                    ms2[:, cs], ones_h[:], sq[:, cs],
                                    start=(kp == 0), stop=(kp == NKP - 1),
                                )
                        lnt2 = sp.tile([P, T], F32, name=f"lnt2_{hb}")
                        nc.scalar.activation(
                            lnt2[:], ms2[:], AF.Ln, bias=eps_sb[:], scale=1.0 / D
                        )
                        nc.scalar.activation(bcast2[:], lnt2[:], AF.Exp, scale=-0.5)

                    ffs = []
                    with ExitStack() as fph:
                        fpp = fph.enter_context(
                            tc.tile_pool(name=f"ffu_ps_{hb}", bufs=1, space="PSUM")
                        )
                        for m in range(NFT):
                            for which, wsrc in (("f", wf_t), ("u", wu_t)):
                                wt = sp.tile(
                                    [P, NKP, P], DT, tag="wffu", bufs=3,
                                    name=f"w{which}_{hb}_{m}",
                                )
                                nc.sync.dma_start(out=wt[:], in_=wsrc[m])
                                ps = fpp.tile(
                                    [P, T], F32, tag=f"ps_{which}", bufs=2,
                                    name=f"ps{which}_{hb}_{m}",
                                )
                                for kp in range(NKP):
                                    for ch in range(T // 512):
                                        cs = slice(ch * 512, (ch + 1) * 512)
                                        nc.tensor.matmul(
                                            ps[:, cs], wt[:, kp, :], xmh[kp][:, cs],
                                            start=(kp == 0), stop=(kp == NKP - 1),
                                        )
                                # fold the norm2 scale into the eviction
                                nt = sp.tile(
                                    [P, T], DT, tag=f"nrm_{which}", bufs=3,
                                    name=f"nt{which}_{hb}_{m}",
                                )
                                nc.vector.scalar_tensor_tensor(
                                    nt[:], ps[:], 1.0, bcast2[:], ALU.mult, ALU.mult
                                )
                                if which == "f":
                                    ft = sp.tile(
                                        [P, T], DT, tag="ffs", bufs=3,
                                        name=f"ff_{hb}_{m}",
                                    )
                                    nc.scalar.activation(ft[:], nt[:], AF.Silu)
                                    ffs.append(ft)
                                else:
                                    ht = bp.tile(
                                        [P, T], DT, tag=f"h{m}", name=f"h_{hb}_{m}"
                                    )
                                    nc.vector.tensor_mul(ht[:], nt[:], ffs[m][:])
                                    hsb.append(ht)

                    # w_out projection + residual, partial output
                    with ExitStack() as oph:
                        opp = oph.enter_context(
                            tc.tile_pool(name=f"wo2_ps_{hb}", bufs=1, space="PSUM")
                        )
                        for dt in range(NDT):
                            wt = sp.tile(
                                [P, NFT, P], DT, tag="wot", bufs=3,
                                name=f"wot_{hb}_{dt}",
                            )
                            nc.sync.dma_start(out=wt[:], in_=wout_t[dt])
                            ps = opp.tile(
                                [P, T], F32, tag="ps_o2", bufs=2,
                                name=f"pso2_{hb}_{dt}",
                            )
                            for m in range(NFT):
                                for ch in range(T // 512):
                                    cs = slice(ch * 512, (ch + 1) * 512)
                                    nc.tensor.matmul(
                                        ps[:, cs], wt[:, m, :], hsb[m][:, cs],
                                        start=(m == 0), stop=(m == NFT - 1),
                                    )
                            ysb = sp.tile(
                                [P, T], F32, tag="ysb", bufs=3, name=f"ysb_{hb}_{dt}"
                            )
                            nc.vector.scalar_tensor_tensor(
                                ysb[:], xmh[dt][:], 1.0 / N_CORES, ps[:],
                                ALU.mult, ALU.add,
                            )
                            nc.sync.dma_start(
                                out=y[dt * P : (dt + 1) * P, bs], in_=ysb[:]
                            )


_NC_CACHE = {}


def _get_nc():
    if "nc" not in _NC_CACHE:
        _NC_CACHE["nc"] = _build()
    return _NC_CACHE["nc"]


def _host_prep(x, sin, cos, attn_norm_w, ff_norm_w, wq, wk, wv, wo, w_ff, w_up, w_out):
    x2 = np.asarray(x, np.float32).reshape(M, D)
    xT = np.ascontiguousarray(x2.T)

    sinT = np.asarray(sin, np.float32).reshape(M, HALF).T
    cosT = np.asarray(cos, np.float32).reshape(M, HALF).T
    cc = np.concatenate([cosT, cosT], axis=0)
    ss = np.concatenate([sinT, sinT], axis=0)
    css = np.stack([cc, ss]).astype(NPDT)

    anw = np.asarray(attn_norm_w, np.float32)[:, None]
    fnw = np.asarray(ff_norm_w, np.float32)[:, None]
    wqn = (anw * np.asarray(wq, np.float32)) * (H ** -0.5)
    wkn = anw * np.asarray(wk, np.float32)
    wvn = anw * np.asarray(wv, np.float32)
    wfn = fnw * np.asarray(w_ff, np.float32)
    wun = fnw * np.asarray(w_up, np.float32)
    wo = np.asarray(wo, np.float32)
    w_out = np.asarray(w_out, np.float32)

    def mtile(w):
        # [K, F] -> [F/P, P, K/P, P] with [m, p, kp, j] = w[kp*P+p, m*P+j]
        K, F = w.shape
        return np.ascontiguousarray(
            w.reshape(K // P, P, F // P, P).transpose(2, 1, 0, 3)
        )

    in_maps = []
    for c in range(N_CORES):
        qs = slice(c * QC, (c + 1) * QC)
        fs = slice(c * FC, (c + 1) * FC)
        in_maps.append(
            {
                "xT_h": xT.astype(NPDT),
                "css": css,
                "wq_t": mtile(wqn[:, qs]).astype(NPDT),
                "wk_t": mtile(wkn[:, qs]).astype(NPDT),
                "wv_n": wvn[:, qs].astype(NPDT),
                # [h, p, dt, j] = wo[c*QC + h*P + p, dt*P + j]
                "wo_t": np.ascontiguousarray(
                    wo[qs, :].reshape(NH, P, NDT, P)
                ).astype(NPDT),
                "wf_t": mtile(wfn[:, fs]).astype(NPDT),
                "wu_t": mtile(wun[:, fs]).astype(NPDT),
                "wout_t": mtile(w_out[fs, :]).astype(NPDT),
            }
        )
    return in_maps


def kernel(**inputs) -> np.ndarray:
    nc = _get_nc()
    in_maps = _host_prep(**inputs)
    res = run_bass_kernel_spmd(
        nc, in_maps, core_ids=list(range(N_CORES)), trace=False
    )
    acc = res.results[0]["y"].astype(np.float64)
    for c in range(1, N_CORES):
        acc += res.results[c]["y"]
    return np.ascontiguousarray(acc.T).astype(np.float32).reshape(B, T, D)
